# revision 25
# baseline (speedup 1.0000x reference)
"""CTRNN kernel for 8x TRN2 NeuronCores (data-parallel over batch).

Reference computation (per timestep t, alpha = dt/tau = 1e-3):
    xp = inputs @ W_in + b_rec                      # [T, B, H]
    h  = (1-a)*h + a*relu(h @ W_rec.T + xp[t])      # recurrence, h0 = 0
    out[t] = h @ W_out + b_out                      # [T, B, O]

Device design (per core, B_local = 32; everything in transposed hT layout,
state columns chunk-major [p=j-in-chunk, (cj, b)]):
  - Per step: 20 bf16 matmuls (4 j-chunks x (1 W_in + 4 W_rec k-chunks))
    accumulate z = a*(W_rec h + W_in x)^T into ONE PSUM bank [128, 128].
    W chunks are stationary (bf16 -> fast weight load); moving N=32.
  - VectorE-only state update (no ScalarE on the cross-step chain):
      ud = (1-a)*u                       (off-chain, overlaps matmuls)
      stage_slot = bf16((z max 0) + ud)  (fused, the only op on the chain)
      u = (z max 0) + ud                 (fp32 master, off-chain)
    b_rec != 0 falls back to per-chunk ScalarE relu with per-partition bias.
  - bf16 h_t lands in one of 8 SBUF staging slots (2 tiles x 4 slots);
    the matmuls of step t+1 read the slot directly.
  - Every 4 steps the completed group is projected inline:
    out[o, (t4, b)] = W_out.T @ stage + b_out (4 matmuls N=128 from SBUF,
    ScalarE eviction with b_out bias, DMA out). No DRAM hs, no phase 2.
  - x^T is DMA-loaded 4 timesteps per transfer.

Host does layout-only prep: batch shard, transpose inputs to [T, I, B],
pre-scale W_rec/W_in/b_rec by alpha, cast to bf16; output is returned as
[O, T*B] per core and transposed/concatenated on the host.
"""

import os
import sys

for _p in ("/opt/trn_rl_repo",):
    if _p not in sys.path:
        sys.path.insert(0, _p)

import numpy as np
import ml_dtypes

import concourse.bass as bass
import concourse.bacc as bacc
import concourse.mybir as mybir
import concourse.tile as tile
from concourse.bass_utils import run_bass_kernel_spmd

BF16_NP = ml_dtypes.bfloat16

# Problem shapes (hardcoded per contract)
T_FULL = 1024
B_FULL = 256
IN_SIZE = 128
H = 512
O = 32
N_CORES = 8
B = B_FULL // N_CORES  # 32 per core

ALPHA = 0.1 / 100.0
DECAY = 1.0 - ALPHA

P = 128
NJ = H // P  # 4 output-row chunks
NK = H // P  # 4 contraction chunks

FP32 = mybir.dt.float32
BF16 = mybir.dt.bfloat16

NT_BLK = 16  # timesteps per phase-2 block -> moving N = NT_BLK * B = 512

LAST_EXEC_NS = None
LAST_RESULTS = None


def build_module(T: int, bias_mode: bool = False):
    """Build the per-core Bass module (same program for all cores).

    bias_mode: when b_rec is nonzero, relu is done per j-chunk with a
    per-partition bias AP (slower tail); otherwise one relu covers all
    four chunks.
    """
    assert T % 8 == 0, T
    # Bacc (not raw Bass): its compile() splits multi-semaphore waits into
    # the 1-wait-per-instruction form the TRN2 ISA requires.
    nc = bacc.Bacc("TRN2", target_bir_lowering=False, debug=False)

    x_d = nc.declare_dram_parameter("x", [T, IN_SIZE, B], BF16, isOutput=False)
    wrec_d = nc.declare_dram_parameter("wrec", [NK, P, H], BF16, isOutput=False)
    win_d = nc.declare_dram_parameter("win", [IN_SIZE, H], BF16, isOutput=False)
    brec_d = nc.declare_dram_parameter("brec", [P, NJ], FP32, isOutput=False)
    wout_d = nc.declare_dram_parameter("wout", [NJ, P, O], BF16, isOutput=False)
    bout_d = nc.declare_dram_parameter("bout", [O, 1], FP32, isOutput=False)
    out_d = nc.declare_dram_parameter("out", [O, T * B], FP32, isOutput=True)

    RELU = mybir.ActivationFunctionType.Relu
    IDENT = mybir.ActivationFunctionType.Identity
    MULT = mybir.AluOpType.mult
    ADD = mybir.AluOpType.add
    MAX = mybir.AluOpType.max

    NGRP = T // 4          # 4 steps per hs flush group
    W = NJ * B             # 128: per-step state width (chunk-major columns)

    with tile.TileContext(nc) as tc:
        with (
            tc.tile_pool(name="const", bufs=1) as cpool,
            tc.tile_pool(name="xin", bufs=4) as xpool,
            tc.tile_pool(name="zpsum", bufs=4, space="PSUM") as zpool,
            tc.tile_pool(name="ph2ps", bufs=2, space="PSUM") as opool_ps,
            tc.tile_pool(name="relu", bufs=4) as rpool,
            tc.tile_pool(name="ud", bufs=3) as udpool,
            tc.tile_pool(name="ph2out", bufs=4) as opool,
        ):
            # ---- constants ----
            w_sb = cpool.tile([P, NK * H], BF16, name="wrec_sb", tag="wrec_sb")
            win_sb = cpool.tile([P, H], BF16, name="win_sb", tag="win_sb")
            brec_sb = cpool.tile([P, NJ], FP32, name="brec_sb", tag="brec_sb")
            wout_sb = cpool.tile([P, NJ * O], BF16, name="wout_sb", tag="wout_sb")
            bout_sb = cpool.tile([O, 1], FP32, name="bout_sb", tag="bout_sb")

            for ck in range(NK):
                nc.sync.dma_start(out=w_sb[:, ck * H:(ck + 1) * H], in_=wrec_d[ck])
            nc.sync.dma_start(out=win_sb[:], in_=win_d[:])
            nc.sync.dma_start(out=brec_sb[:], in_=brec_d[:])
            for cj in range(NJ):
                nc.sync.dma_start(out=wout_sb[:, cj * O:(cj + 1) * O], in_=wout_d[cj])
            nc.sync.dma_start(out=bout_sb[:], in_=bout_d[:])

            # ---- persistent state ----
            # fp32 master state, chunk-major columns [p, (cj, b)]
            u_sb = cpool.tile([P, W], FP32, name="u_sb", tag="u_sb")
            # bf16 state staging: 2 tiles x 4 slots of [128, 128]; slot q = t%8
            # lives in stage[q//4] columns (q%4)*W. Separate tiles so the
            # group DMA (reads one tile) never blocks copies into the other.
            stage = [cpool.tile([P, 4 * W], BF16, name=f"stage{i}", tag=f"stage{i}")
                     for i in range(2)]
            nc.vector.memset(u_sb[:], 0.0)
            nc.vector.memset(stage[1][:, 3 * W:4 * W], 0.0)   # h_0 = 0 (slot 7)

            # Warm-up activation with minimal deps: walrus attaches the ACT
            # table load to the first activation, which costs sync-wait slots
            # that the first real relu (psum deps) does not have.
            warm = cpool.tile([P, 1], FP32, name="act_warm", tag="act_warm")
            nc.vector.memset(warm[:], 0.0)
            nc.scalar.activation(warm[:], warm[:], RELU)

            # ---- recurrence ----
            for t in range(T):
                if t % 4 == 0:
                    # batch-load 4 timesteps of x^T: [p=i, (t:4, b:32)]
                    xt = xpool.tile([P, 4, B], BF16, name="xt", tag="xt")
                    nc.sync.dma_start(out=xt[:],
                                      in_=x_d[t:t + 4].rearrange("t p b -> p t b"))

                # ud = (1-a)*u, computed early (off the cross-step chain)
                ud = udpool.tile([P, W], FP32, name="ud", tag="ud")
                nc.vector.tensor_scalar_mul(ud[:], u_sb[:], DECAY)

                qr = (t - 1) % 8
                rd = stage[qr // 4]
                rd0 = (qr % 4) * W
                qw = t % 8
                wr = stage[qw // 4]
                wr0 = (qw % 4) * W

                z = zpool.tile([P, W], FP32, name="z", tag="z")
                for cj in range(NJ):
                    zc = z[:, cj * B:(cj + 1) * B]
                    nc.tensor.matmul(
                        zc, lhsT=win_sb[:, cj * P:(cj + 1) * P],
                        rhs=xt[:, t % 4, :], start=True, stop=False,
                    )
                    for ck in range(NK):
                        nc.tensor.matmul(
                            zc,
                            lhsT=w_sb[:, ck * H + cj * P: ck * H + (cj + 1) * P],
                            rhs=rd[:, rd0 + ck * B: rd0 + (ck + 1) * B],
                            start=False, stop=(ck == NK - 1),
                        )

                if bias_mode:
                    # general path: relu with per-partition bias on ScalarE
                    r = rpool.tile([P, W], FP32, name="r", tag="r")
                    for cj in range(NJ):
                        nc.scalar.activation(
                            r[:, cj * B:(cj + 1) * B], z[:, cj * B:(cj + 1) * B],
                            RELU, bias=brec_sb[:, cj:cj + 1], scale=1.0)
                    nc.vector.tensor_tensor(wr[:, wr0:wr0 + W], r[:], ud[:], ADD)
                    nc.vector.tensor_tensor(u_sb[:], r[:], ud[:], ADD)
                else:
                    # fast path (b_rec == 0): fused relu+add on VectorE only.
                    # On the cross-step chain: bf16 staging slot first.
                    nc.vector.scalar_tensor_tensor(wr[:, wr0:wr0 + W], z[:], 0.0,
                                                   ud[:], MAX, ADD)
                    # fp32 master state (consumed by next step's ud op)
                    nc.vector.scalar_tensor_tensor(u_sb[:], z[:], 0.0,
                                                   ud[:], MAX, ADD)

                if t % 4 == 3:
                    # inline output projection for the completed 4-step group:
                    # out[o, (t4, b)] = W_out.T @ h + b_out, straight from the
                    # SBUF staging tile (no DRAM roundtrip, no phase 2).
                    po = opool_ps.tile([O, 4 * B], FP32, name="po", tag="po")
                    grp = wr.rearrange("p (s c b) -> p s c b", s=4, c=NJ, b=B)
                    for c in range(NJ):
                        nc.tensor.matmul(
                            po[:], lhsT=wout_sb[:, c * O:(c + 1) * O],
                            rhs=grp[:, :, c, :],
                            start=(c == 0), stop=(c == NJ - 1),
                        )
                    ob = opool.tile([O, 4 * B], FP32, name="ob", tag="ob")
                    nc.scalar.activation(ob[:], po[:], IDENT,
                                         bias=bout_sb[:, 0:1], scale=1.0)
                    nc.sync.dma_start(
                        out=out_d[:, (t - 3) * B:(t + 1) * B], in_=ob[:])

    nc.compile()
    return nc


def _prep_shared(W_rec, W_in, b_rec, W_out, b_out):
    wrecT = (ALPHA * W_rec.T).astype(BF16_NP)            # [k, j]
    wrec_chunks = np.ascontiguousarray(wrecT.reshape(NK, P, H))
    win = np.ascontiguousarray((ALPHA * W_in).astype(BF16_NP))
    brec = np.ascontiguousarray(
        (ALPHA * b_rec.astype(np.float64)).astype(np.float32).reshape(NJ, P).T
    )
    wout = np.ascontiguousarray(W_out.astype(BF16_NP).reshape(NJ, P, O))
    bout = np.ascontiguousarray(b_out.astype(np.float32).reshape(O, 1))
    return wrec_chunks, win, brec, wout, bout


def kernel(inputs, W_rec, W_in, b_rec, W_out, b_out):
    inputs = np.asarray(inputs, dtype=np.float32)
    W_rec = np.asarray(W_rec, dtype=np.float32)
    W_in = np.asarray(W_in, dtype=np.float32)
    b_rec = np.asarray(b_rec, dtype=np.float32)
    W_out = np.asarray(W_out, dtype=np.float32)
    b_out = np.asarray(b_out, dtype=np.float32)
    T = inputs.shape[0]
    nc = build_module(T, bias_mode=bool(np.any(b_rec)))

    wrec_chunks, win, brec, wout, bout = _prep_shared(W_rec, W_in, b_rec, W_out, b_out)

    in_maps = []
    for c in range(N_CORES):
        xc = inputs[:, c * B:(c + 1) * B, :]                       # [T, B, I]
        xT = np.ascontiguousarray(xc.transpose(0, 2, 1)).astype(BF16_NP)  # [T, I, B]
        in_maps.append({
            "x": xT, "wrec": wrec_chunks, "win": win,
            "brec": brec, "wout": wout, "bout": bout,
        })

    trace = bool(int(os.environ.get("KERNEL_TRACE", "0")))
    try:
        kr = run_bass_kernel_spmd(nc, in_maps, list(range(N_CORES)), trace=trace)
    except ModuleNotFoundError:
        kr = run_bass_kernel_spmd(nc, in_maps, list(range(N_CORES)), trace=False)
    global LAST_EXEC_NS, LAST_RESULTS
    LAST_EXEC_NS = kr.exec_time_ns
    LAST_RESULTS = kr
    res = kr.results

    outs = []
    for c in range(N_CORES):
        o = np.asarray(res[c]["out"], dtype=np.float32)            # [O, T*B]
        outs.append(o.reshape(O, T, B).transpose(1, 2, 0))         # [T, B, O]
    return np.concatenate(outs, axis=1)                            # [T, B_FULL, O]
